# revision 47
# baseline (speedup 1.0000x reference)
"""Multi-head self-attention (B=2, S=2048, d_model=1024, H=16, RoPE, causal)
on 8 Trainium2 NeuronCores, tensor-parallel over heads (2 heads/core).

Restructured from the 364us baseline around four findings from its trace:
  - DVE `reciprocal` (8 cyc/elem) cost 63us and stalled the PE at every
    attention-group boundary -> replaced with reciprocal_approx_fast
    (~51 ULP, plenty for a softmax denominator).
  - 33us startup stall from ~45 serial DMA issues -> weight/x loads are
    batched into multi-dim-AP DMAs, ordered so the first q-projection
    matmul can start as soon as wq[k0]+x[k0] land.
  - RoPE ran full-batch after all projections (20us serial DVE bubble)
    -> per-512-token-block RoPE in bf16, hidden under projection matmuls.
  - phases were serial per batch, so the PE idled ~45% and HAM kept
    dropping it to 1.2 GHz -> single fused emission pipeline: attention
    group g of batch b interleaves (as PE filler) the projection blocks
    of the next batch and the output-projection chunks, keeping the PE
    stream dense.

Per-core layout (core c owns heads 2c, 2c+1):
  - host pre-transposes x -> xT [1024, 4096] bf16; per-core transposed
    weight slices; Wq/Wk rows de-interleaved per head ([evens, odds]) so
    RoPE's partner swap is 4 partition-strip copies (SBUF->SBUF DMA).
  - q/k produced transposed ([dim, tok]); V re-transposed to [tok, dim]
    via DMA-xbar transposes (off the PE) with an appended ones-block so
    the P@V matmul also yields the softmax denominator.
  - scores computed transposed per 128-wide k-tile with both heads packed
    in the PE via tile_position (64-row contractions run concurrently);
    exp on the scalar engine straight out of PSUM with the 1/sqrt(64)
    scale folded in; causal masking via a post-exp {0,1} multiply on the
    diagonal 128x128 block only.
  - attention outputs AllGathered per 512-token group (128KB/rank) so the
    gathers overlap compute; each core then computes its 128 output dims
    of the final projection. Host concatenates + transposes.
"""

from collections import deque

import ml_dtypes
import numpy as np

import concourse.bass as bass
import concourse.mybir as mybir
import concourse.tile as tile
from concourse import bacc
from concourse.bass_utils import run_bass_kernel_spmd

P = 128
B, S, D = 2, 2048, 1024
T = B * S          # 4096 flattened tokens
H = 16
DH = 64            # head dim
NC = 8             # cores
HPC = H // NC      # heads per core = 2
DPC = HPC * DH     # dims per core = 128
KT = D // P        # 8 contraction tiles for d_model
TB = 512           # token block for projections
G = 512            # attention q-group width
NG = S // G        # 4 groups per batch
A2W = 1024         # AllToAll chunk width (tokens); 2 chunks per batch
TPC = A2W // NC    # tokens per core per chunk after re-shard = 128
ROPE_THETA = 10000.0

F32 = mybir.dt.float32
BF = mybir.dt.bfloat16

_CACHE = {}


def _build():
    nc = bacc.Bacc(None, target_bir_lowering=False)

    xT = nc.dram_tensor("xT", [D, T], BF, kind="ExternalInput")
    wq = nc.dram_tensor("wq", [D, DPC], BF, kind="ExternalInput")
    wk = nc.dram_tensor("wk", [D, DPC], BF, kind="ExternalInput")
    wv = nc.dram_tensor("wv", [D, DPC], BF, kind="ExternalInput")
    wo = nc.dram_tensor("wo", [D, D], BF, kind="ExternalInput")
    cosb = nc.dram_tensor("cosb", [P, S], BF, kind="ExternalInput")
    sinb = nc.dram_tensor("sinb", [P, S], BF, kind="ExternalInput")
    maskb = nc.dram_tensor("maskb", [P, P], BF, kind="ExternalInput")
    iden = nc.dram_tensor("iden", [P, P], BF, kind="ExternalInput")
    outT = nc.dram_tensor("outT", [D, T // NC], BF, kind="ExternalOutput")

    with tile.TileContext(nc) as tc:
        with (
            tc.tile_pool(name="cst", bufs=1) as cst,
            tc.tile_pool(name="wpool", bufs=1) as wpool,
            tc.tile_pool(name="xin", bufs=8) as xin,
            tc.tile_pool(name="qk", bufs=1) as qkpool,
            tc.tile_pool(name="tmp", bufs=2) as tmp,
            tc.tile_pool(name="pt", bufs=3) as ptpool,
            tc.tile_pool(name="att", bufs=2) as attp,
            tc.tile_pool(name="prj", bufs=2) as prj,
            tc.tile_pool(name="ps", bufs=1, space="PSUM") as ps,
            tc.tile_pool(name="dram", bufs=1, space="DRAM") as dram,
        ):
            # ---- constant loads, ordered so block (0,0) can start ASAP ----
            xTr = xT.rearrange("(ko p) t -> p ko t", p=P)
            ws = {}
            for name, w in (("q", wq), ("k", wk), ("v", wv)):
                ws[name] = wpool.tile([P, KT, DPC], BF, name=f"w{name}")
            ws["o"] = wpool.tile([P, KT, D], BF, name="wo")  # full Wo^T
            cos_t = cst.tile([P, S], BF)
            sin_t = cst.tile([P, S], BF)
            mk01 = cst.tile([P, P], BF)
            idn = cst.tile([P, P], BF)
            wqr = wq.rearrange("(ko p) m -> p ko m", p=P)
            xb00 = xin.tile([P, KT, TB], BF, name="xb", tag="xb", bufs=8)
            nc.scalar.dma_start(idn, iden[:, :])
            for k in range(KT):  # interleave wq / x-block0 per k-tile
                e = nc.sync if k % 2 == 0 else nc.scalar
                e.dma_start(ws["q"][:, k], wqr[:, k])
                e.dma_start(xb00[:, k], xTr[:, k, 0:TB])
                if k == 1:  # only the first-half tables are needed early
                    nc.sync.dma_start(cos_t[:, 0 : S // 2], cosb[:, 0 : S // 2])
                    nc.scalar.dma_start(sin_t[:, 0 : S // 2], sinb[:, 0 : S // 2])
            nc.gpsimd.dma_start(ws["k"], wk.rearrange("(ko p) m -> p ko m", p=P))
            nc.gpsimd.dma_start(ws["v"], wv.rearrange("(ko p) m -> p ko m", p=P))
            nc.sync.dma_start(mk01, maskb[:, :])
            nc.gpsimd.dma_start(cos_t[:, S // 2 :], cosb[:, S // 2 :])
            nc.gpsimd.dma_start(sin_t[:, S // 2 :], sinb[:, S // 2 :])
            # dummy AllToAll: pulls the collective-framework barrier and the
            # RDH cold path into the startup window, so the first real
            # gather runs warm
            dmy_in = dram.tile([P, 16], BF, name="dmy_in")
            dmy_out = dram.tile([P, 16], BF, name="dmy_out")
            scrb = cst.tile([1, 16], BF)
            nc.vector.memset(scrb, 0.0)
            nc.gpsimd.dma_start(dmy_in[0:1, :], scrb)
            nc.gpsimd.collective_compute(
                "AllToAll",
                mybir.AluOpType.bypass,
                replica_groups=[list(range(NC))],
                ins=[dmy_in[:, :]],
                outs=[dmy_out[:, :]],
            )

            # AllToAll re-shard buffers: per (batch, 1024-token half).
            # ag_in rows = 8 shards x (my 128 head-dims); shard j covers
            # chunk-tokens [j*TPC, (j+1)*TPC) and is delivered to core j.
            # ag_out rows = 8 ranks x (their 128 head-dims) = the full
            # 1024 attention dims, in order, for my TPC tokens.
            ag_in = [
                [dram.tile([NC * DPC, TPC], BF, name=f"a2in{b}_{a}") for a in range(2)]
                for b in range(B)
            ]
            ag_out = [
                [dram.tile([NC * DPC, TPC], BF, name=f"a2out{b}_{a}") for a in range(2)]
                for b in range(B)
            ]

            qTs, kTs, vxs = {}, {}, {}

            def units_block(b, j):
                """Generator: project x block j of batch b -> qT/kT (roped)
                and vx tiles; yields between PE units for interleaving."""
                T0 = S * b
                js = slice(j * TB, (j + 1) * TB)
                if j == 0:
                    qTs[b] = qkpool.tile([P, S], BF, name="qT", tag=f"qT{b}")
                    kTs[b] = qkpool.tile([P, S], BF, name="kT", tag=f"kT{b}")
                    vxs[b] = [
                        qkpool.tile([P, S // P, P], BF, name=f"vx{h}", tag=f"vx{b}_{h}")
                        for h in range(HPC)
                    ]
                    # ones-block FIRST so the P@V denominator lands on
                    # partitions 0:64 (reciprocal_approx_fast mishandles
                    # base-partition-shifted inputs)
                    for h in range(HPC):
                        nc.gpsimd.memset(vxs[b][h][:, :, 0:DH], 1.0)
                if (b, j) != (0, 0):
                    xb = xin.tile([P, KT, TB], BF, name="xb", tag="xb", bufs=8)
                    nc.sync.dma_start(
                        xb, xTr[:, :, T0 + j * TB : T0 + (j + 1) * TB]
                    )
                else:
                    xb = xb00
                yield
                for name in ("q", "k", "v"):
                    pp = ps.tile([P, TB], F32, name="pp", tag="pp", bufs=1)
                    for k in range(KT):
                        nc.tensor.matmul(
                            pp,
                            ws[name][:, k],
                            xb[:, k],
                            start=(k == 0),
                            stop=(k == KT - 1),
                        )
                        yield
                    if name == "v":
                        vr = tmp.tile([P, TB], BF, name="vr", tag="vr")
                        nc.vector.tensor_copy(vr, pp)
                        yield
                        vtp = ps.tile(
                            [P, TB // P, P], BF, name="vtp", tag="pp", bufs=1
                        )
                        for t4 in range(TB // P):
                            nc.tensor.transpose(
                                vtp[:, t4], vr[:, t4 * P : (t4 + 1) * P], idn
                            )
                            yield
                        for h in range(HPC):
                            nc.vector.tensor_copy(
                                vxs[b][h][:, 4 * j : 4 * j + 4, DH:P],
                                vtp[:, :, DH * h : DH * (h + 1)],
                            )
                        yield
                    else:
                        raw = tmp.tile([P, TB], BF, name="raw", tag=f"{name}raw")
                        nc.scalar.copy(raw, pp)
                        gsw = tmp.tile([P, TB], BF, name="gsw", tag=f"{name}g")
                        # on the scalar queue, right behind the raw copy that
                        # produces their source (no cross-queue HOL risk)
                        for s0, s1 in ((0, 32), (32, 0), (64, 96), (96, 64)):
                            nc.scalar.dma_start(
                                gsw[s0 : s0 + 32], raw[s1 : s1 + 32]
                            )
                        dstT = qTs[b] if name == "q" else kTs[b]
                        t1 = tmp.tile([P, TB], BF, name="t1", tag="t1")
                        nc.vector.tensor_tensor(
                            t1, raw, cos_t[:, js], mybir.AluOpType.mult
                        )
                        yield
                        t2 = tmp.tile([P, TB], BF, name="t2", tag="t2")
                        nc.vector.tensor_tensor(
                            t2, gsw, sin_t[:, js], mybir.AluOpType.mult
                        )
                        nc.vector.tensor_tensor(
                            dstT[:, js], t1, t2, mybir.AluOpType.add
                        )
                        yield

            def units_outproj(b, a):
                """Generator: full-d_model output projection of my TPC tokens
                for half-chunk a of batch b (after the AllToAll re-shard)."""
                rhs = prj.tile([P, KT, TPC], BF, name="rhs", tag="rhs")
                nc.sync.dma_start(
                    rhs, ag_out[b][a].rearrange("(ko p) t -> p ko t", p=P)
                )
                yield
                obt = prj.tile([P, KT, TPC], BF, name="obt", tag="obt")
                for m in range(D // P):
                    po = ps.tile([P, TPC], F32, name="po", tag="po", bufs=1)
                    for k in range(KT):
                        nc.tensor.matmul(
                            po,
                            ws["o"][:, k, m * P : (m + 1) * P],
                            rhs[:, k],
                            start=(k == 0),
                            stop=(k == KT - 1),
                        )
                        if k % 2 == 1:
                            yield
                    nc.vector.tensor_copy(obt[:, m], po)
                cs = slice((2 * b + a) * TPC, (2 * b + a + 1) * TPC)
                nc.sync.dma_start(
                    outT.rearrange("(m p) t -> p m t", p=P)[:, :, cs], obt
                )
                yield

            fill = deque()

            def pull(n):
                while n > 0 and fill:
                    try:
                        next(fill[0])
                        n -= 1
                    except StopIteration:
                        fill.popleft()

            def exhaust_fill():
                while fill:
                    for _ in fill.popleft():
                        pass

            def attention_qgroup(b, g):
                """Causal attention for q-cols [g*G, (g+1)*G) of batch b."""
                qT, kT, vx = qTs[b], kTs[b], vxs[b]
                oa = ps.tile([P, HPC, G], F32, name="oa", tag="oa", bufs=1)
                n_t = (g + 1) * G // P  # valid k-tiles
                pend = None  # software pipeline: P@V lags scores by one t

                def p_at_v(t, c0, pT):
                    for h in range(HPC):
                        nc.tensor.matmul(
                            oa[:, h, c0:],
                            vx[h][:, t],
                            pT[:, h, c0:],
                            start=(t == 0),
                            stop=(t == n_t - 1),
                            skip_group_check=True,
                        )

                for t in range(n_t):
                    c0 = max(0, t * P - g * G)
                    sc = ps.tile([P, HPC, G], F32, name="sc", tag="sc", bufs=2)
                    for h in range(HPC):
                        hs = slice(DH * h, DH * (h + 1))
                        nc.tensor.matmul(
                            sc[:, h, c0:],
                            kT[hs, t * P : (t + 1) * P],
                            qT[hs, g * G + c0 : (g + 1) * G],
                            start=True,
                            stop=True,
                            tile_position=(DH * h, 0),
                        )
                    pT = ptpool.tile([P, HPC, G], BF, name="pT", tag="pT")
                    nc.scalar.activation(
                        pT[:, :, c0:],
                        sc[:, :, c0:],
                        mybir.ActivationFunctionType.Exp,
                        scale=1.0 / np.sqrt(DH),
                    )
                    if t * P >= g * G:  # diagonal block: causal 0/1 mask
                        for h in range(HPC):
                            nc.vector.tensor_tensor(
                                pT[:, h, c0 : c0 + P],
                                pT[:, h, c0 : c0 + P],
                                mk01,
                                mybir.AluOpType.mult,
                            )
                    if pend is not None:
                        p_at_v(*pend)
                    pend = (t, c0, pT)
                    pull(3)
                p_at_v(*pend)
                rec = attp.tile([DH, HPC, G], F32, name="rec", tag="rec")
                for h in range(HPC):
                    nc.vector.reciprocal_approx_fast(rec[:, h], oa[0:DH, h])
                at = attp.tile([DH, HPC, G], BF, name="at", tag="at")
                for h in range(HPC):
                    nc.vector.tensor_tensor(
                        at[:, h], oa[DH:P, h], rec[:, h], mybir.AluOpType.mult
                    )
                a, gg = divmod(g, 2)
                # scatter my [64, 512] per-head slab into 4 shard blocks:
                # dst row = (4*gg + s)*128 + 64*h + p, col = token within shard
                agt = ag_in[b][a]
                for h in range(HPC):
                    dst = bass.AP(
                        tensor=agt[:, :].tensor,
                        offset=gg * 4 * P * TPC + DH * h * TPC,
                        ap=[[TPC, DH], [P * TPC, 4], [1, TPC]],
                    )
                    nc.gpsimd.dma_start(dst, at[:, h])
                if gg == 1:
                    nc.gpsimd.collective_compute(
                        "AllToAll",
                        mybir.AluOpType.bypass,
                        replica_groups=[list(range(NC))],
                        ins=[ag_in[b][a][:, :]],
                        outs=[ag_out[b][a][:, :]],
                    )

            def exhaust(gen):
                for _ in gen:
                    pass

            # ---- fused emission pipeline ----
            # ACT table warm-up: force the EXP table load during startup DMAs
            scr = cst.tile([1, 16], F32)
            nc.vector.memset(scr, 1.0)
            nc.scalar.activation(
                scr, scr, mybir.ActivationFunctionType.Exp, scale=0.0
            )
            # Batches' attention groups interleave so one batch's group-end
            # ladder (recip/at-mult/AG) hides under the other's compute, and
            # the AllGathers spread across the kernel instead of bunching.
            exhaust(units_block(0, 0))
            fill.append(units_block(0, 1))
            attention_qgroup(0, 0)
            exhaust_fill()
            # full Wo (2MB, needed ~110us in): late so it doesn't steal
            # HBM bandwidth from the startup-critical loads
            nc.gpsimd.dma_start(ws["o"], wo.rearrange("(ko p) m -> p ko m", p=P))
            fill.append(units_block(0, 2))
            fill.append(units_block(1, 0))
            attention_qgroup(0, 1)  # A2A(0,0) fires at its end
            exhaust_fill()
            fill.append(units_block(1, 1))
            attention_qgroup(1, 0)
            exhaust_fill()
            fill.append(units_block(0, 3))
            attention_qgroup(0, 2)
            exhaust_fill()
            fill.append(units_block(1, 2))
            attention_qgroup(1, 1)  # A2A(1,0) fires
            exhaust_fill()
            fill.append(units_block(1, 3))
            fill.append(units_outproj(0, 0))
            attention_qgroup(0, 3)  # A2A(0,1) fires
            exhaust_fill()
            attention_qgroup(1, 2)
            exhaust_fill()
            fill.append(units_outproj(1, 0))
            fill.append(units_outproj(0, 1))
            attention_qgroup(1, 3)  # A2A(1,1) fires
            exhaust_fill()
            exhaust(units_outproj(1, 1))

    nc.compile()
    return nc


def _host_inputs(x, token_positions, Wq, Wk, Wv, Wo):
    xT = np.ascontiguousarray(x.reshape(T, D).T).astype(ml_dtypes.bfloat16)  # [D, T]

    # de-interleave perm within each 64-dim head: [evens, odds]
    perm = np.concatenate(
        [64 * h + np.r_[np.arange(0, 64, 2), np.arange(1, 64, 2)] for h in range(HPC)]
    )

    pos = token_positions.astype(np.float64)  # [S]
    inv_freq = ROPE_THETA ** (-np.arange(0, DH, 2, dtype=np.float64) / DH)  # [32]
    ang = pos[:, None] * inv_freq[None, :]  # [S, 32]
    cos = np.cos(ang).T.astype(np.float32)  # [32, S]
    sin = np.sin(ang).T.astype(np.float32)
    cosb = np.concatenate([cos, cos, cos, cos], axis=0).astype(ml_dtypes.bfloat16)
    sinb = np.concatenate([-sin, sin, -sin, sin], axis=0).astype(ml_dtypes.bfloat16)

    maskb = np.triu(np.ones((P, P), dtype=np.float32)).astype(ml_dtypes.bfloat16)
    iden = np.eye(P, dtype=np.float32).astype(ml_dtypes.bfloat16)

    woT = np.ascontiguousarray(Wo.T).astype(ml_dtypes.bfloat16)  # [in, out]
    in_maps = []
    for c in range(NC):
        rs = slice(DPC * c, DPC * (c + 1))
        in_maps.append(
            {
                "xT": xT,
                "wq": np.ascontiguousarray(Wq[rs][perm].T).astype(ml_dtypes.bfloat16),
                "wk": np.ascontiguousarray(Wk[rs][perm].T).astype(ml_dtypes.bfloat16),
                "wv": np.ascontiguousarray(Wv[rs].T).astype(ml_dtypes.bfloat16),
                "wo": woT,
                "cosb": cosb,
                "sinb": sinb,
                "maskb": maskb,
                "iden": iden,
            }
        )
    return in_maps


def kernel(x, token_positions, Wq, Wk, Wv, Wo, _trace=False, _result=[None]):
    x = np.asarray(x, dtype=np.float32)
    token_positions = np.asarray(token_positions)
    Wq, Wk, Wv, Wo = (np.asarray(w, dtype=np.float32) for w in (Wq, Wk, Wv, Wo))

    if "nc" not in _CACHE:
        _CACHE["nc"] = _build()
    nc = _CACHE["nc"]

    in_maps = _host_inputs(x, token_positions, Wq, Wk, Wv, Wo)
    res = run_bass_kernel_spmd(nc, in_maps, core_ids=list(range(NC)), trace=_trace)
    _result[0] = res
    out = np.empty((B, S, D), dtype=np.float32)
    for c in range(NC):
        r = np.asarray(res.results[c]["outT"], dtype=np.float32)  # [D, 4*TPC]
        for b in range(B):
            for a in range(2):
                ts = a * A2W + c * TPC
                out[b, ts : ts + TPC, :] = r[:, (2 * b + a) * TPC : (2 * b + a + 1) * TPC].T
    return out
